# revision 39
# baseline (speedup 1.0000x reference)
"""Trainium2 Bass kernel for nn_MHInrAttn (sparse_attention, b=4 s=1024 f=1024 h=16).

Strategy (8 NeuronCores):
  - The reference uses a raw .reshape(b, h, s, d_h) with NO transpose, so head h's
    Q/K/V data comes from ROWS [64h, 64h+64) of the projected [s, f] matrix.
    Sharding 2 heads per core means each core only needs 128 rows of x per batch.
  - The first softmax (over masked str_mat) depends only on the inputs, so the
    host computes it and ships exp(sm)^T in bf16, packed to the causal region,
    with the q axis PERMUTED to q' = 64c + r (q = 16r + c) so the on-device Q^T
    assembly from PE transposes is a contiguous copy. The device computes
    E = exp(qk) * exp(sm) -- exp straight off the QK PSUM, then a 2x bf16 DVE
    multiply, keeping PSUM slot hold times short.
  - Per core: project Q/K/V for its 128 rows (all 4 batches) in bf16, run
    attention for its 2 heads x 4 batches in scores^T [k, q'] orientation, and
    produce a bf16 partial output projection (its heads' rows of Wo).
  - Host: shard inputs, run SPMD on 8 cores, sum the 8 bf16 partials in fp32,
    un-permute the q axis, transpose, add bo.

Scheduling: every engine executes its instruction stream IN ORDER with a
4-deep dependency-wait window, so cross-phase overlap is software-scheduled:
attention steps of batch b are interleaved at emission time with the output
projection of batch b-1 and the projections/transposes of batch b+1, keeping
the PE (the roofline engine at ~128us) fed. V rides a DMA shuffle through
DRAM; bias adds ride the Pool-engine PSUM->SBUF copies against partition-
broadcast bias tiles; the second softmax's row sums come free from a ones
column in V; 1/rowsum is broadcast with a tiny fp32r PE matmul.
"""

import numpy as np

B, S, F, H, D = 4, 1024, 1024, 16, 64
NCORES = 8
HPC = H // NCORES  # heads per core
P = 128

# packed causal sm^T: per (b, hp, partition k) chunks j=0..7 of [16 c, 64-8j r]
_CHUNK_W = [64 - 8 * j for j in range(8)]
_CHUNK_CUM = np.cumsum([0] + [16 * w for w in _CHUNK_W]).tolist()
PPACK = _CHUNK_CUM[8]  # per-partition packed elems = 4608

# q' = 64c + r  <->  q = 16r + c
_QP = np.arange(1024)
QMAP = (16 * (_QP % 64) + _QP // 64).astype(np.int64)  # q of q'

_CACHE = {}


def _build_nc(causal=True):
    from contextlib import ExitStack

    import concourse.bacc as bacc
    import concourse.tile as tile
    from concourse import mybir

    dt = mybir.dt
    f32 = dt.float32
    bf16 = dt.bfloat16
    Exp = mybir.ActivationFunctionType.Exp
    Alu = mybir.AluOpType

    ppack = PPACK if causal else 8192
    cum = _CHUNK_CUM if causal else [1024 * j for j in range(9)]

    nc = bacc.Bacc("TRN2", target_bir_lowering=False, debug=False)

    xT_d = nc.dram_tensor("xT", [B, P, 8 * P], bf16, kind="ExternalInput").ap()
    sm_d = nc.dram_tensor("smT", [B, HPC, P, ppack], bf16, kind="ExternalInput").ap()
    wq_d = nc.dram_tensor("wq", [F, F], bf16, kind="ExternalInput").ap()
    wk_d = nc.dram_tensor("wk", [F, F], bf16, kind="ExternalInput").ap()
    wv_d = nc.dram_tensor("wv", [F, F], bf16, kind="ExternalInput").ap()
    wo_d = nc.dram_tensor("wo", [P, F], bf16, kind="ExternalInput").ap()
    bias_d = nc.dram_tensor("bqkv", [3, F], bf16, kind="ExternalInput").ap()
    ident_d = nc.dram_tensor("ident", [P, P], bf16, kind="ExternalInput").ap()
    out_d = nc.dram_tensor("outT", [B, F, S], bf16, kind="ExternalOutput").ap()

    def f32r(ap):
        return ap.bitcast(dt.float32r)

    with ExitStack() as ctx:
        tc = ctx.enter_context(tile.TileContext(nc))
        consts = ctx.enter_context(tc.tile_pool(name="consts", bufs=1))
        wpool = ctx.enter_context(tc.tile_pool(name="wpool", bufs=1))
        qtkt = ctx.enter_context(tc.tile_pool(name="qtkt", bufs=1))
        v2p = ctx.enter_context(tc.tile_pool(name="v2", bufs=1))
        outp = ctx.enter_context(tc.tile_pool(name="outp", bufs=1))
        dramp = ctx.enter_context(tc.tile_pool(name="dram", bufs=2, space="DRAM"))

        xtp = ctx.enter_context(tc.tile_pool(name="xt", bufs=1))
        qkvcp = ctx.enter_context(tc.tile_pool(name="qkvc", bufs=2))
        smp = ctx.enter_context(tc.tile_pool(name="smp", bufs=2))
        epool = ctx.enter_context(tc.tile_pool(name="ep", bufs=8))
        miscp = ctx.enter_context(tc.tile_pool(name="misc", bufs=1))
        osp = ctx.enter_context(tc.tile_pool(name="os", bufs=3))

        # PSUM budget (16KB/partition): qk [128,512]f32 x3 (QK scores, proj,
        # r2bc) + pv 4 tags x1 (PV accum, outproj) + tp 1 = 16KB
        qkps = ctx.enter_context(tc.tile_pool(name="qkps", bufs=3, space="PSUM"))
        pvps = ctx.enter_context(tc.tile_pool(name="pvps", bufs=1, space="PSUM"))
        tpps = ctx.enter_context(tc.tile_pool(name="tpps", bufs=1, space="PSUM"))

        # ---------- consts + global loads ----------
        ident = consts.tile([P, P], bf16, tag="ident", name="ident")
        nc.sync.dma_start(out=ident, in_=ident_d)
        ones_f = consts.tile([P, 64], bf16, tag="ones", name="ones")
        nc.vector.memset(ones_f, 1.0)
        bias_sb = {}
        for t_i in range(3):
            bias_sb[t_i] = consts.tile([1, F], bf16, tag=f"bias{t_i}",
                                       name=f"bias{t_i}")
            nc.sync.dma_start(out=bias_sb[t_i], in_=bias_d[t_i:t_i + 1, :])
        xt = {}
        xt[0] = xtp.tile([P, 8, P], bf16, tag="xt0", name="xt0")
        nc.sync.dma_start(out=xt[0], in_=xT_d[0].rearrange("p (kc r) -> p kc r", kc=8))
        # W loads split 4-way so they land fast across DMA queues (V first:
        # the first projection chain is V)
        wt = {}
        for t_i, w_d in ((2, wv_d), (0, wq_d), (1, wk_d)):
            wt[t_i] = wpool.tile([P, 8, F], bf16, tag=f"w{t_i}", name=f"w{t_i}")
            for q4 in range(4):
                nc.sync.dma_start(
                    out=wt[t_i][:, 2 * q4:2 * q4 + 2, :],
                    in_=w_d[256 * q4:256 * (q4 + 1), :].rearrange(
                        "(kc p) f -> p kc f", p=P))
        wo_sb = consts.tile([P, F], bf16, tag="wo", name="wo")
        nc.sync.dma_start(out=wo_sb, in_=wo_d)
        for b in range(1, B):
            xt[b] = xtp.tile([P, 8, P], bf16, tag=f"xt{b}", name=f"xt{b}")
            nc.sync.dma_start(out=xt[b],
                              in_=xT_d[b].rearrange("p (kc r) -> p kc r", kc=8))

        bias_bc = {}
        for t_i in range(3):
            bb = consts.tile([P, F], bf16, tag=f"bbc{t_i}", name=f"bbc{t_i}")
            bias_bc[t_i] = bb
            nc.gpsimd.partition_broadcast(bb, bias_sb[t_i][0:1, :])

        QT, KT, V2, OT = {}, {}, {}, {}
        for b in range(B):
            QT[b] = qtkt.tile([P, S], bf16, tag=f"qt{b}", name=f"qt{b}")
            KT[b] = qtkt.tile([P, S], bf16, tag=f"kt{b}", name=f"kt{b}")
            OT[b] = outp.tile([P, S], bf16, tag=f"ot{b}", name=f"ot{b}")
            for hp in range(HPC):
                V2[b, hp] = v2p.tile([P, 8, P], bf16, tag=f"v{b}{hp}", name=f"v{b}{hp}")

        smt = {}

        # ---------- emission units (software pipelining) ----------
        def u_smt(b):
            def f():
                for hp in range(HPC):
                    smt[b, hp] = smp.tile([P, ppack], bf16, tag=f"sm{hp}",
                                          name=f"sm{b}{hp}")
                    nc.sync.dma_start(out=smt[b, hp], in_=sm_d[b, hp])
            return f

        def u_proj(b, t_i, h2, qkvc):
            def f():
                if h2 == 0:
                    qkvc[t_i] = qkvcp.tile([P, F], bf16, tag=f"c{t_i}",
                                           name=f"c{t_i}{b}")
                cc = qkvc[t_i]
                ps = qkps.tile([P, 512], f32, tag="qk", name="pj")
                for kc in range(8):
                    nc.tensor.matmul(
                        ps, xt[b][:, kc, :],
                        wt[t_i][:, kc, 512 * h2:512 * (h2 + 1)],
                        start=(kc == 0), stop=(kc == 7))
                nc.vector.scalar_tensor_tensor(
                    cc[:, 512 * h2:512 * (h2 + 1)], ps, 1.0,
                    bias_bc[t_i][:, 512 * h2:512 * (h2 + 1)],
                    Alu.mult, Alu.add)
            return f

        def u_vshuffle(b, qkvc):
            def f():
                vs = dramp.tile([P, F], bf16, tag="vs", name=f"vs{b}")
                nc.sync.dma_start(out=vs, in_=qkvc[2][:])
                for hp in range(HPC):
                    nc.gpsimd.memset(V2[b, hp], 0.0)
                    dcol = 64 * hp
                    ones_col = 64 if hp == 0 else 0
                    src = vs[64 * hp:64 * hp + 64, :].rearrange(
                        "(j r) (cb d) -> (r cb) j d", j=8, cb=16)
                    nc.sync.dma_start(out=V2[b, hp][:, :, dcol:dcol + 64], in_=src)
                    nc.gpsimd.memset(V2[b, hp][:, :, ones_col:ones_col + 1], 1.0)
            return f

        def u_transpose(b, t_i, half, hp, qkvc):
            # pst[d, 64*cb8 + r] = qkvc[64hp + r, 64*cb + d]
            # Q keeps transpose-natural order: QT[d, q'=64c+r] (contiguous)
            # K is shuffled to natural key order: KT[d, k=16r+c] (strided)
            def f():
                cc = qkvc[t_i]
                base = 64 * hp
                pst = tpps.tile([64, 512], bf16, tag="tp", name="tp")
                for cb8 in range(8):
                    cb = 8 * half + cb8
                    nc.tensor.transpose(
                        pst[0:64, 64 * cb8:64 * cb8 + 64],
                        cc[base:base + 64, 64 * cb:64 * cb + 64],
                        ident[base:base + 64, base:base + 64])
                dstmap = QT if t_i == 0 else KT
                dst64 = dstmap[b][64 * hp:64 * hp + 64, :]
                if t_i == 0:
                    nc.vector.tensor_copy(
                        dst64[:, 512 * half:512 * (half + 1)], pst[0:64, :])
                else:
                    dst = dst64.rearrange(
                        "p (r cb) -> p cb r", cb=16)[:, 8 * half:8 * half + 8, :]
                    nc.vector.tensor_copy(
                        dst, pst[0:64, :].rearrange("p (cb8 r) -> p cb8 r", cb8=8))
            return f

        def u_norm(b, hp, h2, pv):
            # normalize rows of PV by 1/rowsum (from the ones column)
            def f():
                sum_row = 64 if hp == 0 else 0
                dlo = 64 * hp
                sl = slice(512 * h2, 512 * (h2 + 1))
                r2sb = miscp.tile([P, S], bf16, tag="r2sb", name="r2sb")
                with nc.allow_low_precision(reason="1/rowsum broadcast via fp32r"):
                    nc.vector.reciprocal(r2sb[sum_row:sum_row + 1, sl],
                                         pv[hp, h2][sum_row:sum_row + 1, :])
                psb = qkps.tile([P, 512], f32, tag="qk", name="bc")
                nc.tensor.matmul(psb[dlo:dlo + 64, :],
                                 ones_f[sum_row:sum_row + 1, :],
                                 r2sb[sum_row:sum_row + 1, sl],
                                 start=True, stop=True)
                # engines may read only one PSUM operand: stage r2bc in SBUF
                r2bc = miscp.tile([P, 512], f32, tag="r2bc", name="r2bc")
                nc.scalar.copy(r2bc[dlo:dlo + 64, :], psb[dlo:dlo + 64, :])
                nc.vector.tensor_mul(OT[b][dlo:dlo + 64, sl],
                                     pv[hp, h2][dlo:dlo + 64, :],
                                     r2bc[dlo:dlo + 64, :])
            return f

        def u_outproj(b, fo):
            # partial output projection; PSUM rides the freed pv slots,
            # copies rotate Act/DVE/Pool
            def f():
                ot = osp.tile([P, S], bf16, tag="os", name="os")
                for h2 in range(2):
                    k = 2 * fo + h2
                    ops = qkps.tile([P, 512], f32, tag="qk", name="op")
                    nc.tensor.matmul(ops,
                                     wo_sb[:, 128 * fo:128 * (fo + 1)],
                                     OT[b][:, 512 * h2:512 * (h2 + 1)],
                                     start=True, stop=True)
                    dst = ot[:, 512 * h2:512 * (h2 + 1)]
                    if k % 2 == 0:
                        nc.scalar.copy(dst, ops)
                    else:
                        nc.vector.tensor_copy(dst, ops)
                nc.sync.dma_start(out=out_d[b, 128 * fo:128 * (fo + 1), :], in_=ot)
            return f

        def units_P(b, qkvc):
            # transposes spread between projection chains so the single-slot
            # transpose PSUM never serializes back-to-back groups
            return [
                u_proj(b, 2, 0, qkvc), u_proj(b, 2, 1, qkvc), u_vshuffle(b, qkvc),
                u_proj(b, 0, 0, qkvc), u_proj(b, 0, 1, qkvc),
                u_transpose(b, 0, 0, 0, qkvc),
                u_proj(b, 1, 0, qkvc),
                u_transpose(b, 0, 0, 1, qkvc),
                u_proj(b, 1, 1, qkvc),
                u_transpose(b, 0, 1, 0, qkvc),
                u_transpose(b, 1, 0, 0, qkvc),
                u_transpose(b, 0, 1, 1, qkvc),
                u_transpose(b, 1, 0, 1, qkvc),
                u_transpose(b, 1, 1, 0, qkvc),
                u_transpose(b, 1, 1, 1, qkvc),
                u_smt(b),
            ]

        def units_norm(b, pv):
            return [u_norm(b, hp, h2, pv)
                    for hp in range(HPC) for h2 in range(2)]

        def units_out(b):
            return [u_outproj(b, fo) for fo in range(8)]

        # ---------- interleaved emission ----------
        qkvc_of = {0: {}}
        u_smt(0)()
        for u in units_P(0, qkvc_of[0]):
            u()

        pv_of = {}
        for b in range(B):
            # norms(b-1) first (they free the pv slots), then next batch's
            # projections (QT/KT must land before its attention), then
            # outproj(b-1) (only needs OT, can run late)
            other = []
            if b > 0:
                other += units_norm(b - 1, pv_of[b - 1])
            if b + 1 < B:
                qkvc_of[b + 1] = {}
                other += units_P(b + 1, qkvc_of[b + 1])
            if b > 0:
                other += units_out(b - 1)

            pv = {}
            pv_of[b] = pv
            for hp in range(HPC):
                for h2 in range(2):
                    pv[hp, h2] = pvps.tile([P, 512], f32, tag=f"pv{hp}{h2}",
                                           name=f"pv{b}{hp}{h2}")
            # attention steps: per j emit QK/exp/mul for the 4 (hp, h2) chains
            # (interleaved with "other" units), then the previous step's PVs.
            n_steps = 8
            # front-load: next batch's projections/transposes must land
            # before its attention starts
            quota = [0] * 8
            for i in range(len(other)):
                quota[min(i * 8 // max(len(other), 1), 7) if False else (i % 8 if False else min(i // 4, 7))] += 1
            prev = []
            for j in range(8):
                roff = 8 * j if causal else 0
                w = 64 - roff
                cur = []
                for hp in range(HPC):
                    base = 64 * hp
                    for h2 in range(2):
                        qk = qkps.tile([P, 512], f32, tag="qk", name="qk")
                        nc.tensor.matmul(
                            qk,
                            KT[b][base:base + 64, 128 * j:128 * (j + 1)],
                            QT[b][base:base + 64, 512 * h2:512 * (h2 + 1)],
                            start=True, stop=True)
                        Ej = epool.tile([P, 512], bf16, tag="E", name="E")
                        nc.scalar.activation(Ej, qk, Exp)
                        e_sl = Ej.rearrange("p (c r) -> p c r", c=8)[:, :, roff:]
                        sm_sl = smt[b, hp][:, cum[j]:cum[j] + 16 * w].rearrange(
                            "p (c r) -> p c r", c=16)[:, 8 * h2:8 * h2 + 8, :]
                        eng = nc.vector if hp == 0 else nc.gpsimd
                        eng.tensor_mul(e_sl, e_sl, sm_sl)
                        cur.append((hp, h2, Ej))
                    # a slice of cross-batch work between the two heads' QKs
                    take = quota[j] - quota[j] // 2 if hp == 0 else quota[j] // 2
                    for _ in range(take):
                        if other:
                            other.pop(0)()
                for hp, h2, Ej in prev:
                    nc.tensor.matmul(
                        pv[hp, h2], V2[b, hp][:, j - 1, :],
                        Ej[:, 512 * h2:512 * (h2 + 1)],
                        start=(j - 1 == 0), stop=False)
                prev = cur
            for hp, h2, Ej in prev:
                nc.tensor.matmul(
                    pv[hp, h2], V2[b, hp][:, 7, :],
                    Ej[:, 512 * h2:512 * (h2 + 1)],
                    start=False, stop=True)
            for u in other:
                u()

        for u in units_norm(B - 1, pv_of[B - 1]) + units_out(B - 1):
            u()

    nc.compile()
    return nc


def _prep_host(x, str_mat, attn_mask, Wq, bq, Wk, bk, Wv, bv, Wo, bo):
    from concourse import mybir

    bf16 = mybir.dt.np(mybir.dt.bfloat16)

    x = np.asarray(x, np.float32)
    attn_mask = np.asarray(attn_mask, np.float32)
    mask = attn_mask[:, 0]  # [b, s, s]
    causal = bool((mask == np.tril(np.ones((S, S), np.float32))[None]).all())

    # first softmax on host: sm = softmax(where(mask==0, -inf, str), axis=-1)
    str_mat = np.asarray(str_mat, np.float32)
    ez = np.exp(str_mat)
    ez *= (mask != 0.0)[:, None]
    ez /= ez.sum(-1, keepdims=True)
    # [b, h, k, q'] with q permuted to q' (q = QMAP[q'])
    smT = np.ascontiguousarray(ez.transpose(0, 1, 3, 2)[..., QMAP])
    del ez

    # pack per (b, h, k-partition): concat over j of [16 c, 64-8j r],
    # then exponentiate (device computes E = exp(qk) * exp(sm))
    if causal:
        pk = np.empty((B, H, P, PPACK), np.float32)
        for j in range(8):
            ch = smT[:, :, 128 * j:128 * (j + 1), :].reshape(B, H, P, 16, 64)
            pk[:, :, :, _CHUNK_CUM[j]:_CHUNK_CUM[j + 1]] = (
                ch[:, :, :, :, 8 * j:].reshape(B, H, P, -1))
    else:
        pk = smT.reshape(B, H, 8, P, 1024).transpose(0, 1, 3, 2, 4).reshape(
            B, H, P, 8192)
    del smT
    pk = np.exp(pk).astype(bf16)

    # x^T [b, f, s]; per-core slices get pre-shuffled to [b, p, (kc r)]
    # (xt[p, kc, r] = x[b, 128c + r, 128 kc + p]) so the device DMA is trivial
    xT = x.transpose(0, 2, 1)
    Wq_s = (np.asarray(Wq, np.float32) / D).astype(bf16)
    bias = np.stack([np.asarray(bq, np.float32) / D,
                     np.asarray(bk, np.float32),
                     np.asarray(bv, np.float32)]).astype(bf16)
    Wk_h = np.asarray(Wk, np.float32).astype(bf16)
    Wv_h = np.asarray(Wv, np.float32).astype(bf16)
    Wo_h = np.asarray(Wo, np.float32).astype(bf16)
    ident = np.eye(P, dtype=np.float32).astype(bf16)
    in_maps = []
    for c in range(NCORES):
        in_maps.append({
            "xT": np.ascontiguousarray(
                xT[:, :, P * c:P * (c + 1)].reshape(B, 8, P, P)
                .transpose(0, 2, 1, 3).reshape(B, P, 8 * P)).astype(bf16),
            "smT": np.ascontiguousarray(pk[:, HPC * c:HPC * (c + 1)]),
            "wq": Wq_s, "wk": Wk_h, "wv": Wv_h,
            "wo": np.ascontiguousarray(Wo_h[P * c:P * (c + 1)]),
            "bqkv": bias, "ident": ident,
        })
    return in_maps, causal


def kernel(**inputs):
    from concourse.bass_utils import run_bass_kernel_spmd

    in_maps, causal = _prep_host(**inputs)
    if causal not in _CACHE:
        _CACHE[causal] = _build_nc(causal=causal)
    nc = _CACHE[causal]
    res = run_bass_kernel_spmd(nc, in_maps, core_ids=list(range(NCORES)))
    out = np.zeros((B, F, S), np.float32)  # [b, f, q']
    for r in res.results:
        out += np.asarray(r["outT"], np.float32)
    out_t = out.transpose(0, 2, 1)  # [b, q', f]
    res_s = np.empty_like(out_t)
    res_s[:, QMAP, :] = out_t  # q' -> s
    res_s += np.asarray(inputs["bo"], np.float32)
    return np.ascontiguousarray(res_s.astype(np.float32))
